# revision 15
# baseline (speedup 1.0000x reference)
"""Causal scaled-dot-product attention on 8 Trainium2 NeuronCores.

Problem: B=2, H=16, S=2048, D=64, fp32, causal mask.
Sharding: batch*heads (32) split 4-per-core across 8 cores; no collectives.

Per-core bass/Tile kernel, processing heads in pairs (head A on SBUF
partitions 0-63, head B on 64-127 so their K=64 matmuls occupy disjoint
PE row groups and run concurrently):

Phase 1 (per k-chunk row ci, both heads):
  - S^T[k, q] = (K^T)^T @ Q^T on PE (fp16), full causal span
    q in [128ci, S), in pieces of <=PIECE_W columns (PSUM).
  - P^T = exp(scale * S^T) on ScalarE (PSUM -> persistent SBUF fp16,
    causally packed). No max-subtraction needed (scores bounded).
  - Diagonal 128x128 tile: GPSIMD affine_select zeroes P^T where k > q.

Phase 2 (interleaved, q-tile qt = ci just produced):
  - O[q, 0:64] and l=O[q, 64] accumulated in PSUM [128, 65] over chunks
    ci<=qt: matmul(P^T chunk stationary, V_aug moving), V_aug = [V | 1].
  - DVE reciprocal of l + per-partition scale -> O/l, DMA out.

Host packs Q/K transposed (head pairs stacked on partitions) and V
chunked with a ones column, fp16; every DMA is fully contiguous.
"""

import sys
import numpy as np
from contextlib import ExitStack

B, H, S, D = 2, 16, 2048, 64
N_CORES = 8
HEADS_PER_CORE = (B * H) // N_CORES  # 4
CH = 128             # k-chunk (partition tile)
PIECE_W = 512        # S^T piece width per head (1 PSUM bank; A/B paired)
SCALE = 1.0 / np.sqrt(D)
MM_DTYPE = "float16"     # matmul operand dtype (fp16 streams 1 col/cycle;
                         # float32r is precision-safest but 2x slower)
_NP_MM = {"float16": np.float16, "bfloat16": None, "float32r": np.float32,
          "float32": np.float32}

for _p in ("/opt/trn_rl_repo", "/opt/pypackages"):
    if _p not in sys.path:
        sys.path.append(_p)


def _row_off(ci, s_len):
    # packed column offset of causal row ci: sum_{j<ci} (s_len - 128*j)
    return s_len * ci - CH * (ci * (ci - 1)) // 2


def _build_program(n_heads, s_len, piece_w=PIECE_W, mm_dtype=MM_DTYPE):
    import concourse.bass as bass  # noqa: F401
    import concourse.bacc as bacc
    import concourse.tile as tile
    from concourse import mybir

    f32 = mybir.dt.float32
    mmdt = getattr(mybir.dt, mm_dtype)
    n_chunks = s_len // CH
    n_pairs = (n_heads + 1) // 2
    DP1 = D + 1
    pt_len = _row_off(n_chunks, s_len)  # packed P^T length per head

    nc = bacc.Bacc(
        "TRN2",
        target_bir_lowering=False,
        debug=False,
        num_devices=N_CORES,
    )

    qk_d = nc.dram_tensor("qk", [128, n_pairs, 2, s_len], mmdt, kind="ExternalInput").ap()
    v_d = nc.dram_tensor("v", [128, n_heads, n_chunks, DP1], mmdt, kind="ExternalInput").ap()
    o_d = nc.dram_tensor("o", [n_heads, 128, n_chunks * D], f32, kind="ExternalOutput").ap()

    with tile.TileContext(nc) as tc, ExitStack() as ctx:
        const = ctx.enter_context(tc.tile_pool(name="const", bufs=1))
        sb_pt = ctx.enter_context(tc.tile_pool(name="ptp", bufs=2))
        sb_o = ctx.enter_context(tc.tile_pool(name="osb", bufs=2))
        sb_r = ctx.enter_context(tc.tile_pool(name="rsb", bufs=4))
        st_banks = -(-2 * piece_w * 4 // 2048)   # PSUM banks per paired S tile
        ps_s = ctx.enter_context(
            tc.tile_pool(name="pss", bufs=max(1, 6 // st_banks), space="PSUM"))
        ps_o = ctx.enter_context(tc.tile_pool(name="pso", bufs=2, space="PSUM"))

        qk = const.tile([128, n_pairs, 2, s_len], mmdt)
        v = const.tile([128, n_heads, n_chunks, DP1], mmdt)
        for pair in range(n_pairs):
            nc.sync.dma_start(out=qk[:, pair], in_=qk_d[:, pair])
            for hh in (2 * pair, 2 * pair + 1):
                if hh < n_heads:
                    nc.sync.dma_start(out=v[:, hh], in_=v_d[:, hh])

        def ph1_row(pair, heads, ci, pt_pair):
            """mm1 pieces (heads A/B alternating -> concurrent PE row
            groups) + one exp per piece covering both heads + diag mask."""
            sp0 = CH * ci
            span = s_len - sp0
            ro = _row_off(ci, s_len)
            for poff in range(0, span, piece_w):
                w = min(piece_w, span - poff)
                # [128, 2, piece_w]: each head's slice is one PSUM bank
                st = ps_s.tile([128, 2, piece_w], f32, tag="st")
                for idx, hh in enumerate(heads):
                    bp = 64 * (hh % 2)
                    nc.tensor.matmul(
                        st[:, idx, 0:w],
                        qk[bp:bp + 64, pair, 1, sp0:sp0 + CH],
                        qk[bp:bp + 64, pair, 0, sp0 + poff:sp0 + poff + w],
                        start=True,
                        stop=True,
                    )
                nc.scalar.activation(
                    pt_pair[:, 0:len(heads), ro + poff:ro + poff + w],
                    st[:, 0:len(heads), 0:w],
                    mybir.ActivationFunctionType.Exp,
                    scale=float(SCALE),
                )
                if poff == 0:
                    for idx in range(len(heads)):
                        nc.gpsimd.affine_select(
                            out=pt_pair[:, idx, ro:ro + CH],
                            in_=pt_pair[:, idx, ro:ro + CH],
                            compare_op=mybir.AluOpType.is_ge,
                            fill=0.0,
                            base=0,
                            pattern=[[1, CH]],
                            channel_multiplier=-1,
                        )

        def ph2_chain(hh, idx, qt, pt_pair, o_stage):
            """accumulate O[q-tile qt] over chunks ci<=qt, normalize."""
            op = ps_o.tile([128, DP1], f32, tag="op")
            for ci in range(qt + 1):
                sl = _row_off(ci, s_len) + CH * (qt - ci)
                nc.tensor.matmul(
                    op,
                    pt_pair[:, idx, sl:sl + CH],
                    v[:, hh, ci, :],
                    start=(ci == 0),
                    stop=(ci == qt),
                )
            r = sb_r.tile([128, 1], f32)
            nc.vector.reciprocal(r, op[:, D:D + 1])
            nc.vector.tensor_scalar_mul(
                o_stage[:, D * qt:D * (qt + 1)], op[:, 0:D], r
            )

        # Software pipeline: chains lag LAG rows behind ph1 so they never
        # wait on a fresh exp, and the tail chains of pair p interleave
        # with pair p+1's first rows (keeps ScalarE fed at the boundary).
        LAG = 3
        pending = []   # deferred chain/DMA closures from the previous pair

        def emit_pending(k):
            for _ in range(min(k, len(pending))):
                pending.pop(0)()

        for pair in range(n_pairs):
            heads = [hh for hh in (2 * pair, 2 * pair + 1) if hh < n_heads]
            pt_pair = sb_pt.tile([128, 2, pt_len], mmdt, tag="ptfull", name=f"ptp{pair}")
            stages = {hh: sb_o.tile([128, n_chunks * D], f32, name=f"ostage{hh}") for hh in heads}

            def chain_unit(hh, idx, qt, pt_pair=pt_pair, stages=stages, heads=heads):
                def run():
                    ph2_chain(hh, idx, qt, pt_pair, stages[hh])
                    if qt == n_chunks // 2 - 1:
                        nc.sync.dma_start(
                            out=o_d[hh][:, 0:(n_chunks // 2) * D],
                            in_=stages[hh][:, 0:(n_chunks // 2) * D],
                        )
                    elif qt == n_chunks - 1:
                        nc.sync.dma_start(
                            out=o_d[hh][:, (n_chunks // 2) * D:],
                            in_=stages[hh][:, (n_chunks // 2) * D:],
                        )
                return run

            for ci in range(n_chunks):
                ph1_row(pair, heads, ci, pt_pair)
                for idx, hh in enumerate(heads):
                    pending.append(chain_unit(hh, idx, ci))
                emit_pending(len(pending) - 2 * LAG)
        emit_pending(len(pending))

    nc.compile()
    return nc


_PROGRAM_CACHE = {}


def _get_program(n_heads=HEADS_PER_CORE, s_len=S, piece_w=PIECE_W, mm_dtype=MM_DTYPE):
    key = (n_heads, s_len, piece_w, mm_dtype)
    if key not in _PROGRAM_CACHE:
        _PROGRAM_CACHE[key] = _build_program(n_heads, s_len, piece_w, mm_dtype)
    return _PROGRAM_CACHE[key]


def _np_mm_dtype(mm_dtype=MM_DTYPE):
    d = _NP_MM.get(mm_dtype)
    if d is None:
        import ml_dtypes
        d = np.dtype(ml_dtypes.bfloat16)
    return d


def _pack_core(Qf, Kf, Vf, heads, s_len=S, mm_dtype=MM_DTYPE):
    """Build the per-core input dict. Qf/Kf/Vf: [B*H, S, D] float32."""
    dt_np = _np_mm_dtype(mm_dtype)
    n_heads = len(heads)
    n_pairs = (n_heads + 1) // 2
    n_chunks = s_len // CH
    qk = np.zeros((128, n_pairs, 2, s_len), dt_np)
    v = np.ones((128, n_heads, n_chunks, D + 1), dt_np)
    for i, hf in enumerate(heads):
        pair, side = divmod(i, 2)
        bp = 64 * side
        qk[bp:bp + 64, pair, 0] = Qf[hf].T
        qk[bp:bp + 64, pair, 1] = Kf[hf].T
        v[:, i, :, :D] = Vf[hf].reshape(n_chunks, CH, D).transpose(1, 0, 2)
    return {"qk": qk, "v": v}


def _unpack_core(o_np, s_len=S):
    """o_np: [n_heads, 128, n_chunks*D] -> [n_heads, S, D]."""
    n_heads = o_np.shape[0]
    n_chunks = s_len // CH
    o = o_np.reshape(n_heads, 128, n_chunks, D)
    return o.transpose(0, 2, 1, 3).reshape(n_heads, s_len, D)


def kernel(Q, K, V, mask):
    Q = np.asarray(Q, np.float32)
    K = np.asarray(K, np.float32)
    V = np.asarray(V, np.float32)
    mask = np.asarray(mask)

    if not np.array_equal(mask, np.tril(np.ones((S, S), dtype=bool))):
        # Non-causal mask: not expected for this problem; numpy fallback.
        scores = np.einsum("bhqd,bhkd->bhqk", Q, K) * SCALE
        scores = np.where(mask, scores, -np.inf)
        scores -= scores.max(-1, keepdims=True)
        p = np.exp(scores)
        p /= p.sum(-1, keepdims=True)
        return np.einsum("bhqk,bhkd->bhqd", p, V).astype(np.float32)

    from concourse.bass_utils import run_bass_kernel_spmd

    Qf = Q.reshape(B * H, S, D)
    Kf = K.reshape(B * H, S, D)
    Vf = V.reshape(B * H, S, D)

    nc = _get_program()
    in_maps = [
        _pack_core(Qf, Kf, Vf, list(range(c * HEADS_PER_CORE, (c + 1) * HEADS_PER_CORE)))
        for c in range(N_CORES)
    ]
    res = run_bass_kernel_spmd(nc, in_maps, core_ids=list(range(N_CORES)))
    out = np.empty((B * H, S, D), np.float32)
    for c in range(N_CORES):
        out[c * HEADS_PER_CORE:(c + 1) * HEADS_PER_CORE] = _unpack_core(res.results[c]["o"])
    return out.reshape(B, H, S, D)


# revision 16
# speedup vs baseline: 1.2004x; 1.2004x over previous
"""Causal scaled-dot-product attention on 8 Trainium2 NeuronCores.

Problem: B=2, H=16, S=2048, D=64, fp32, causal mask.
Sharding: batch*heads (32) split 4-per-core across 8 cores; no collectives.

Per-core bass/Tile kernel, processing heads in pairs (head A on SBUF
partitions 0-63, head B on 64-127 so their K=64 matmuls occupy disjoint
PE row groups and run concurrently):

Phase 1 (per k-chunk row ci, both heads):
  - S^T[k, q] = (K^T)^T @ Q^T on PE (fp16), full causal span
    q in [128ci, S), in pieces of <=PIECE_W columns (PSUM).
  - P^T = exp(scale * S^T) on ScalarE (PSUM -> persistent SBUF fp16,
    causally packed). No max-subtraction needed (scores bounded).
  - Diagonal 128x128 tile: GPSIMD affine_select zeroes P^T where k > q.

Phase 2 (interleaved, q-tile qt = ci just produced):
  - O[q, 0:64] and l=O[q, 64] accumulated in PSUM [128, 65] over chunks
    ci<=qt: matmul(P^T chunk stationary, V_aug moving), V_aug = [V | 1].
  - DVE reciprocal of l + per-partition scale -> O/l, DMA out.

Host packs Q/K transposed (head pairs stacked on partitions) and V
chunked with a ones column, fp16; every DMA is fully contiguous.
"""

import sys
import numpy as np
from contextlib import ExitStack

B, H, S, D = 2, 16, 2048, 64
N_CORES = 8
HEADS_PER_CORE = (B * H) // N_CORES  # 4
CH = 128             # k-chunk (partition tile)
PIECE_W = 512        # S^T piece width per head (1 PSUM bank; A/B paired)
SCALE = 1.0 / np.sqrt(D)
MM_DTYPE = "float16"     # matmul operand dtype (fp16 streams 1 col/cycle;
                         # float32r is precision-safest but 2x slower)
_NP_MM = {"float16": np.float16, "bfloat16": None, "float32r": np.float32,
          "float32": np.float32}

for _p in ("/opt/trn_rl_repo", "/opt/pypackages"):
    if _p not in sys.path:
        sys.path.append(_p)


def _row_off(ci, s_len):
    # packed column offset of causal row ci: sum_{j<ci} (s_len - 128*j)
    return s_len * ci - CH * (ci * (ci - 1)) // 2


def _build_program(n_heads, s_len, piece_w=PIECE_W, mm_dtype=MM_DTYPE):
    import concourse.bass as bass  # noqa: F401
    import concourse.bacc as bacc
    import concourse.tile as tile
    from concourse import mybir

    f32 = mybir.dt.float32
    mmdt = getattr(mybir.dt, mm_dtype)
    n_chunks = s_len // CH
    n_pairs = (n_heads + 1) // 2
    DP1 = D + 1
    pt_len = _row_off(n_chunks, s_len)  # packed P^T length per head

    nc = bacc.Bacc(
        "TRN2",
        target_bir_lowering=False,
        debug=False,
        num_devices=N_CORES,
    )

    qk_d = nc.dram_tensor("qk", [128, n_pairs, 2, s_len], mmdt, kind="ExternalInput").ap()
    v_d = nc.dram_tensor("v", [128, n_heads, n_chunks, DP1], mmdt, kind="ExternalInput").ap()
    o_d = nc.dram_tensor("o", [n_heads, 128, n_chunks * D], f32, kind="ExternalOutput").ap()

    with tile.TileContext(nc) as tc, ExitStack() as ctx:
        const = ctx.enter_context(tc.tile_pool(name="const", bufs=1))
        sb_pt = ctx.enter_context(tc.tile_pool(name="ptp", bufs=2))
        sb_o = ctx.enter_context(tc.tile_pool(name="osb", bufs=2))
        sb_r = ctx.enter_context(tc.tile_pool(name="rsb", bufs=4))
        st_banks = -(-2 * piece_w * 4 // 2048)   # PSUM banks per paired S tile
        ps_s = ctx.enter_context(
            tc.tile_pool(name="pss", bufs=max(1, 6 // st_banks), space="PSUM"))
        ps_o = ctx.enter_context(tc.tile_pool(name="pso", bufs=2, space="PSUM"))

        qk = const.tile([128, n_pairs, 2, s_len], mmdt)
        v = const.tile([128, n_heads, n_chunks, DP1], mmdt)
        for pair in range(n_pairs):
            nc.sync.dma_start(out=qk[:, pair], in_=qk_d[:, pair])
            for hh in (2 * pair, 2 * pair + 1):
                if hh < n_heads:
                    nc.sync.dma_start(out=v[:, hh], in_=v_d[:, hh])

        def ph1_row(pair, heads, ci, pt_pair):
            """mm1 pieces (heads A/B alternating -> concurrent PE row
            groups) + one exp per piece covering both heads + diag mask."""
            sp0 = CH * ci
            span = s_len - sp0
            ro = _row_off(ci, s_len)
            for poff in range(0, span, piece_w):
                w = min(piece_w, span - poff)
                # [128, 2, piece_w]: each head's slice is one PSUM bank
                st = ps_s.tile([128, 2, piece_w], f32, tag="st")
                for idx, hh in enumerate(heads):
                    bp = 64 * (hh % 2)
                    nc.tensor.matmul(
                        st[:, idx, 0:w],
                        qk[bp:bp + 64, pair, 1, sp0:sp0 + CH],
                        qk[bp:bp + 64, pair, 0, sp0 + poff:sp0 + poff + w],
                        start=True,
                        stop=True,
                    )
                nc.scalar.activation(
                    pt_pair[:, 0:len(heads), ro + poff:ro + poff + w],
                    st[:, 0:len(heads), 0:w],
                    mybir.ActivationFunctionType.Exp,
                    scale=float(SCALE),
                )
                if poff == 0:
                    for idx in range(len(heads)):
                        nc.gpsimd.affine_select(
                            out=pt_pair[:, idx, ro:ro + CH],
                            in_=pt_pair[:, idx, ro:ro + CH],
                            compare_op=mybir.AluOpType.is_ge,
                            fill=0.0,
                            base=0,
                            pattern=[[1, CH]],
                            channel_multiplier=-1,
                        )

        def ph2_chain(hh, idx, qt, pt_pair, o_stage):
            """accumulate O[q-tile qt] over chunks ci<=qt, normalize."""
            op = ps_o.tile([128, DP1], f32, tag="op")
            for ci in range(qt + 1):
                sl = _row_off(ci, s_len) + CH * (qt - ci)
                nc.tensor.matmul(
                    op,
                    pt_pair[:, idx, sl:sl + CH],
                    v[:, hh, ci, :],
                    start=(ci == 0),
                    stop=(ci == qt),
                )
            r = sb_r.tile([128, 1], f32)
            nc.vector.reciprocal(r, op[:, D:D + 1])
            nc.vector.tensor_scalar_mul(
                o_stage[:, D * qt:D * (qt + 1)], op[:, 0:D], r
            )

        # Software pipeline: chains lag LAG rows behind ph1 so they never
        # wait on a fresh exp, and the tail chains of pair p interleave
        # with pair p+1's first rows (keeps ScalarE fed at the boundary).
        LAG = 2
        pending = []   # deferred chain/DMA closures from the previous pair

        def emit_pending(k):
            for _ in range(min(k, len(pending))):
                pending.pop(0)()

        for pair in range(n_pairs):
            heads = [hh for hh in (2 * pair, 2 * pair + 1) if hh < n_heads]
            pt_pair = sb_pt.tile([128, 2, pt_len], mmdt, tag="ptfull", name=f"ptp{pair}")
            stages = {hh: sb_o.tile([128, n_chunks * D], f32, name=f"ostage{hh}") for hh in heads}

            def chain_unit(hh, idx, qt, pt_pair=pt_pair, stages=stages, heads=heads):
                def run():
                    ph2_chain(hh, idx, qt, pt_pair, stages[hh])
                    if qt == n_chunks // 2 - 1:
                        nc.sync.dma_start(
                            out=o_d[hh][:, 0:(n_chunks // 2) * D],
                            in_=stages[hh][:, 0:(n_chunks // 2) * D],
                        )
                    elif qt == n_chunks - 1:
                        nc.sync.dma_start(
                            out=o_d[hh][:, (n_chunks // 2) * D:],
                            in_=stages[hh][:, (n_chunks // 2) * D:],
                        )
                return run

            for ci in range(n_chunks):
                ph1_row(pair, heads, ci, pt_pair)
                for idx, hh in enumerate(heads):
                    pending.append(chain_unit(hh, idx, ci))
                emit_pending(len(pending) - 2 * LAG)
        emit_pending(len(pending))

    nc.compile()
    return nc


_PROGRAM_CACHE = {}


def _get_program(n_heads=HEADS_PER_CORE, s_len=S, piece_w=PIECE_W, mm_dtype=MM_DTYPE):
    key = (n_heads, s_len, piece_w, mm_dtype)
    if key not in _PROGRAM_CACHE:
        _PROGRAM_CACHE[key] = _build_program(n_heads, s_len, piece_w, mm_dtype)
    return _PROGRAM_CACHE[key]


def _np_mm_dtype(mm_dtype=MM_DTYPE):
    d = _NP_MM.get(mm_dtype)
    if d is None:
        import ml_dtypes
        d = np.dtype(ml_dtypes.bfloat16)
    return d


def _pack_core(Qf, Kf, Vf, heads, s_len=S, mm_dtype=MM_DTYPE):
    """Build the per-core input dict. Qf/Kf/Vf: [B*H, S, D] float32."""
    dt_np = _np_mm_dtype(mm_dtype)
    n_heads = len(heads)
    n_pairs = (n_heads + 1) // 2
    n_chunks = s_len // CH
    qk = np.zeros((128, n_pairs, 2, s_len), dt_np)
    v = np.ones((128, n_heads, n_chunks, D + 1), dt_np)
    for i, hf in enumerate(heads):
        pair, side = divmod(i, 2)
        bp = 64 * side
        qk[bp:bp + 64, pair, 0] = Qf[hf].T
        qk[bp:bp + 64, pair, 1] = Kf[hf].T
        v[:, i, :, :D] = Vf[hf].reshape(n_chunks, CH, D).transpose(1, 0, 2)
    return {"qk": qk, "v": v}


def _unpack_core(o_np, s_len=S):
    """o_np: [n_heads, 128, n_chunks*D] -> [n_heads, S, D]."""
    n_heads = o_np.shape[0]
    n_chunks = s_len // CH
    o = o_np.reshape(n_heads, 128, n_chunks, D)
    return o.transpose(0, 2, 1, 3).reshape(n_heads, s_len, D)


def kernel(Q, K, V, mask):
    Q = np.asarray(Q, np.float32)
    K = np.asarray(K, np.float32)
    V = np.asarray(V, np.float32)
    mask = np.asarray(mask)

    if not np.array_equal(mask, np.tril(np.ones((S, S), dtype=bool))):
        # Non-causal mask: not expected for this problem; numpy fallback.
        scores = np.einsum("bhqd,bhkd->bhqk", Q, K) * SCALE
        scores = np.where(mask, scores, -np.inf)
        scores -= scores.max(-1, keepdims=True)
        p = np.exp(scores)
        p /= p.sum(-1, keepdims=True)
        return np.einsum("bhqk,bhkd->bhqd", p, V).astype(np.float32)

    from concourse.bass_utils import run_bass_kernel_spmd

    Qf = Q.reshape(B * H, S, D)
    Kf = K.reshape(B * H, S, D)
    Vf = V.reshape(B * H, S, D)

    nc = _get_program()
    in_maps = [
        _pack_core(Qf, Kf, Vf, list(range(c * HEADS_PER_CORE, (c + 1) * HEADS_PER_CORE)))
        for c in range(N_CORES)
    ]
    res = run_bass_kernel_spmd(nc, in_maps, core_ids=list(range(N_CORES)))
    out = np.empty((B * H, S, D), np.float32)
    for c in range(N_CORES):
        out[c * HEADS_PER_CORE:(c + 1) * HEADS_PER_CORE] = _unpack_core(res.results[c]["o"])
    return out.reshape(B, H, S, D)


# revision 17
# speedup vs baseline: 1.2329x; 1.0271x over previous
"""Causal scaled-dot-product attention on 8 Trainium2 NeuronCores.

Problem: B=2, H=16, S=2048, D=64, fp32, causal mask.
Sharding: batch*heads (32) split 4-per-core across 8 cores; no collectives.

Per-core bass/Tile kernel, processing heads in pairs (head A on SBUF
partitions 0-63, head B on 64-127 so their K=64 matmuls occupy disjoint
PE row groups and run concurrently):

Phase 1 (per k-chunk row ci, both heads):
  - S^T[k, q] = (K^T)^T @ Q^T on PE (fp16), full causal span
    q in [128ci, S), in pieces of <=PIECE_W columns (PSUM).
  - P^T = exp(scale * S^T) on ScalarE (PSUM -> persistent SBUF fp16,
    causally packed). No max-subtraction needed (scores bounded).
  - Diagonal 128x128 tile: GPSIMD affine_select zeroes P^T where k > q.

Phase 2 (interleaved, q-tile qt = ci just produced):
  - O[q, 0:64] and l=O[q, 64] accumulated in PSUM [128, 65] over chunks
    ci<=qt: matmul(P^T chunk stationary, V_aug moving), V_aug = [V | 1].
  - DVE reciprocal of l + per-partition scale -> O/l, DMA out.

Host packs Q/K transposed (head pairs stacked on partitions) and V
chunked with a ones column, fp16; every DMA is fully contiguous.
"""

import sys
import numpy as np
from contextlib import ExitStack

B, H, S, D = 2, 16, 2048, 64
N_CORES = 8
HEADS_PER_CORE = (B * H) // N_CORES  # 4
CH = 128             # k-chunk (partition tile)
PIECE_W = 512        # S^T piece width per head (1 PSUM bank; A/B paired)
SCALE = 1.0 / np.sqrt(D)
MM_DTYPE = "float16"     # matmul operand dtype (fp16 streams 1 col/cycle;
                         # float32r is precision-safest but 2x slower)
_NP_MM = {"float16": np.float16, "bfloat16": None, "float32r": np.float32,
          "float32": np.float32}

for _p in ("/opt/trn_rl_repo", "/opt/pypackages"):
    if _p not in sys.path:
        sys.path.append(_p)


def _row_off(ci, s_len):
    # packed column offset of causal row ci: sum_{j<ci} (s_len - 128*j)
    return s_len * ci - CH * (ci * (ci - 1)) // 2


def _build_program(n_heads, s_len, piece_w=PIECE_W, mm_dtype=MM_DTYPE):
    import concourse.bass as bass  # noqa: F401
    import concourse.bacc as bacc
    import concourse.tile as tile
    from concourse import mybir

    f32 = mybir.dt.float32
    mmdt = getattr(mybir.dt, mm_dtype)
    n_chunks = s_len // CH
    n_pairs = (n_heads + 1) // 2
    DP1 = D + 1
    pt_len = _row_off(n_chunks, s_len)  # packed P^T length per head

    nc = bacc.Bacc(
        "TRN2",
        target_bir_lowering=False,
        debug=False,
        num_devices=N_CORES,
    )

    qk_d = nc.dram_tensor("qk", [128, n_pairs, 2, s_len], mmdt, kind="ExternalInput").ap()
    v_d = nc.dram_tensor("v", [128, n_heads, n_chunks, DP1], mmdt, kind="ExternalInput").ap()
    o_d = nc.dram_tensor("o", [n_heads, 128, n_chunks * D], f32, kind="ExternalOutput").ap()

    with tile.TileContext(nc) as tc, ExitStack() as ctx:
        const = ctx.enter_context(tc.tile_pool(name="const", bufs=1))
        sb_pt = ctx.enter_context(tc.tile_pool(name="ptp", bufs=2))
        sb_o = ctx.enter_context(tc.tile_pool(name="osb", bufs=2))
        sb_r = ctx.enter_context(tc.tile_pool(name="rsb", bufs=4))
        st_banks = -(-2 * piece_w * 4 // 2048)   # PSUM banks per paired S tile
        ps_s = ctx.enter_context(
            tc.tile_pool(name="pss", bufs=max(1, 6 // st_banks), space="PSUM"))
        ps_o = ctx.enter_context(tc.tile_pool(name="pso", bufs=2, space="PSUM"))

        qk = const.tile([128, n_pairs, 2, s_len], mmdt)
        v = const.tile([128, n_heads, n_chunks, DP1], mmdt)
        for pair in range(n_pairs):
            nc.sync.dma_start(out=qk[:, pair], in_=qk_d[:, pair])
            for hh in (2 * pair, 2 * pair + 1):
                if hh < n_heads:
                    nc.sync.dma_start(out=v[:, hh], in_=v_d[:, hh])

        def ph1_row(pair, heads, ci, pt_pair):
            """mm1 pieces (heads A/B alternating -> concurrent PE row
            groups) + one exp per piece covering both heads + diag mask."""
            sp0 = CH * ci
            span = s_len - sp0
            ro = _row_off(ci, s_len)
            for poff in range(0, span, piece_w):
                w = min(piece_w, span - poff)
                # [128, 2, piece_w]: each head's slice is one PSUM bank
                st = ps_s.tile([128, 2, piece_w], f32, tag="st")
                for idx, hh in enumerate(heads):
                    bp = 64 * (hh % 2)
                    nc.tensor.matmul(
                        st[:, idx, 0:w],
                        qk[bp:bp + 64, pair, 1, sp0:sp0 + CH],
                        qk[bp:bp + 64, pair, 0, sp0 + poff:sp0 + poff + w],
                        start=True,
                        stop=True,
                    )
                nc.scalar.activation(
                    pt_pair[:, 0:len(heads), ro + poff:ro + poff + w],
                    st[:, 0:len(heads), 0:w],
                    mybir.ActivationFunctionType.Exp,
                    scale=float(SCALE),
                )
                if poff == 0:
                    for idx in range(len(heads)):
                        nc.gpsimd.affine_select(
                            out=pt_pair[:, idx, ro:ro + CH],
                            in_=pt_pair[:, idx, ro:ro + CH],
                            compare_op=mybir.AluOpType.is_ge,
                            fill=0.0,
                            base=0,
                            pattern=[[1, CH]],
                            channel_multiplier=-1,
                        )

        def ph2_chain(hh, idx, qt, pt_pair, o_stage):
            """accumulate O[q-tile qt] over chunks ci<=qt, normalize."""
            op = ps_o.tile([128, DP1], f32, tag="op")
            for ci in range(qt + 1):
                sl = _row_off(ci, s_len) + CH * (qt - ci)
                nc.tensor.matmul(
                    op,
                    pt_pair[:, idx, sl:sl + CH],
                    v[:, hh, ci, :],
                    start=(ci == 0),
                    stop=(ci == qt),
                )
            r = sb_r.tile([128, 1], f32)
            nc.vector.reciprocal(r, op[:, D:D + 1])
            nc.vector.tensor_scalar_mul(
                o_stage[:, D * qt:D * (qt + 1)], op[:, 0:D], r
            )

        # Software pipeline: all pairs' rows interleaved (so the PE always
        # has independent work when one pair waits on exp), with chains
        # lagging LAG rows behind ph1 so they never wait on a fresh exp.
        LAG = 2
        pending = []   # deferred chain closures

        def emit_pending(k):
            for _ in range(min(k, len(pending))):
                pending.pop(0)()

        pair_heads = {p: [hh for hh in (2 * p, 2 * p + 1) if hh < n_heads]
                      for p in range(n_pairs)}
        pts = {p: sb_pt.tile([128, 2, pt_len], mmdt, tag="ptfull", name=f"ptp{p}")
               for p in range(n_pairs)}
        stages = {hh: sb_o.tile([128, n_chunks * D], f32, name=f"ostage{hh}")
                  for hh in range(n_heads)}

        def chain_unit(hh, idx, qt, pt_pair):
            def run():
                ph2_chain(hh, idx, qt, pt_pair, stages[hh])
                if qt == n_chunks // 2 - 1:
                    nc.sync.dma_start(
                        out=o_d[hh][:, 0:(n_chunks // 2) * D],
                        in_=stages[hh][:, 0:(n_chunks // 2) * D],
                    )
                elif qt == n_chunks - 1:
                    nc.sync.dma_start(
                        out=o_d[hh][:, (n_chunks // 2) * D:],
                        in_=stages[hh][:, (n_chunks // 2) * D:],
                    )
            return run

        chains_per_row = sum(len(v) for v in pair_heads.values())
        for ci in range(n_chunks):
            for pair in range(n_pairs):
                ph1_row(pair, pair_heads[pair], ci, pts[pair])
                for idx, hh in enumerate(pair_heads[pair]):
                    pending.append(chain_unit(hh, idx, ci, pts[pair]))
            emit_pending(len(pending) - LAG * chains_per_row)
        emit_pending(len(pending))

    nc.compile()
    return nc


_PROGRAM_CACHE = {}


def _get_program(n_heads=HEADS_PER_CORE, s_len=S, piece_w=PIECE_W, mm_dtype=MM_DTYPE):
    key = (n_heads, s_len, piece_w, mm_dtype)
    if key not in _PROGRAM_CACHE:
        _PROGRAM_CACHE[key] = _build_program(n_heads, s_len, piece_w, mm_dtype)
    return _PROGRAM_CACHE[key]


def _np_mm_dtype(mm_dtype=MM_DTYPE):
    d = _NP_MM.get(mm_dtype)
    if d is None:
        import ml_dtypes
        d = np.dtype(ml_dtypes.bfloat16)
    return d


def _pack_core(Qf, Kf, Vf, heads, s_len=S, mm_dtype=MM_DTYPE):
    """Build the per-core input dict. Qf/Kf/Vf: [B*H, S, D] float32."""
    dt_np = _np_mm_dtype(mm_dtype)
    n_heads = len(heads)
    n_pairs = (n_heads + 1) // 2
    n_chunks = s_len // CH
    qk = np.zeros((128, n_pairs, 2, s_len), dt_np)
    v = np.ones((128, n_heads, n_chunks, D + 1), dt_np)
    for i, hf in enumerate(heads):
        pair, side = divmod(i, 2)
        bp = 64 * side
        qk[bp:bp + 64, pair, 0] = Qf[hf].T
        qk[bp:bp + 64, pair, 1] = Kf[hf].T
        v[:, i, :, :D] = Vf[hf].reshape(n_chunks, CH, D).transpose(1, 0, 2)
    return {"qk": qk, "v": v}


def _unpack_core(o_np, s_len=S):
    """o_np: [n_heads, 128, n_chunks*D] -> [n_heads, S, D]."""
    n_heads = o_np.shape[0]
    n_chunks = s_len // CH
    o = o_np.reshape(n_heads, 128, n_chunks, D)
    return o.transpose(0, 2, 1, 3).reshape(n_heads, s_len, D)


def kernel(Q, K, V, mask):
    Q = np.asarray(Q, np.float32)
    K = np.asarray(K, np.float32)
    V = np.asarray(V, np.float32)
    mask = np.asarray(mask)

    if not np.array_equal(mask, np.tril(np.ones((S, S), dtype=bool))):
        # Non-causal mask: not expected for this problem; numpy fallback.
        scores = np.einsum("bhqd,bhkd->bhqk", Q, K) * SCALE
        scores = np.where(mask, scores, -np.inf)
        scores -= scores.max(-1, keepdims=True)
        p = np.exp(scores)
        p /= p.sum(-1, keepdims=True)
        return np.einsum("bhqk,bhkd->bhqd", p, V).astype(np.float32)

    from concourse.bass_utils import run_bass_kernel_spmd

    Qf = Q.reshape(B * H, S, D)
    Kf = K.reshape(B * H, S, D)
    Vf = V.reshape(B * H, S, D)

    nc = _get_program()
    in_maps = [
        _pack_core(Qf, Kf, Vf, list(range(c * HEADS_PER_CORE, (c + 1) * HEADS_PER_CORE)))
        for c in range(N_CORES)
    ]
    res = run_bass_kernel_spmd(nc, in_maps, core_ids=list(range(N_CORES)))
    out = np.empty((B * H, S, D), np.float32)
    for c in range(N_CORES):
        out[c * HEADS_PER_CORE:(c + 1) * HEADS_PER_CORE] = _unpack_core(res.results[c]["o"])
    return out.reshape(B, H, S, D)
